# revision 21
# baseline (speedup 1.0000x reference)
"""GRACE contrastive loss kernel for Trainium2 (8 NeuronCores, SPMD).

Strategy (row-block data parallel):
  - Shard the N=8192 nodes across 8 cores (1024 rows each).
  - Each core projects its z1/z2 block through the 2-layer MLP (fp16 matmuls,
    fp32 accum), computes per-node 1/norms as exp(-0.5*ln(sum h^2)) on the
    activation engine (avoids slow 1-lane reciprocals; Ln/Exp batched so the
    ACT table switches only twice per view), quantizes the normalized
    embeddings to fp8e4, and AllGathers them (fp8, 4.2 MB/view) so every
    core holds full gathered n1/n2 [512, 8192] in SBUF.
  - Similarities run as fp8 DoubleRow matmuls (K=256 per step, 2 steps) in
    [128 x 2048] PSUM groups (double-buffered, 8 banks) with fused exp(2*s)
    + row-sum on the scalar engine (accum_out).  Steady state is jointly
    scalar/PE bound at ~2.2us per 2048-column group.
  - Only S11, S12, S22 are computed explicitly; S21's row sums (= column
    sums of exp(S12) over the full matrix) come from a DVE fp16 2x
    accumulation of the exp(S12) tiles into a [128, 8192] buffer, fp16
    ones-matmul partition reductions interleaved into the first four S22
    row-tiles (borrowing a sim-psum slot), and a ReduceScatter(add) that
    overlaps the back half of S22.
  - The positive diagonal s12_ii is computed exactly in fp32 from h1/h2.
  - Per-core scalar partial out; host sums partials / N.

Measured: 352.9 us HW exec (baseline 718.9 us), rel err 1.2e-05.
"""

import math
import sys

import numpy as np

sys.path.insert(0, "/opt/trn_rl_repo")

import concourse.bass as bass  # noqa: E402
import concourse.mybir as mybir  # noqa: E402
import concourse.tile as tile  # noqa: E402
from concourse import bacc  # noqa: E402
from concourse.bass_utils import run_bass_kernel_spmd  # noqa: E402

F32 = mybir.dt.float32
F32R = mybir.dt.float32r
F16 = mybir.dt.float16
F8 = mybir.dt.float8e4
AF = mybir.ActivationFunctionType
ALU = mybir.AluOpType
DR = mybir.MatmulPerfMode.DoubleRow

N_CORES = 8
N = 8192
D = 512            # feature dim (= H = P in the reference MLP)
NB = N // N_CORES  # 1024 rows per core
KT = D // 128      # 4 k-subtiles
MT = NB // 128     # 8 row tiles per core
NCHUNK = 512
GW = 2048          # sim column-group width (4 psum banks)
NG = N // GW       # 4 groups per row tile
TAU_INV = 2.0      # 1 / tau
E2 = float(np.exp(2.0, dtype=np.float64))  # exp(diag(refl_sim)/tau), diag == 1

TRACE = False
LAST_EXEC_NS = None
_CACHE = {}


def _build_program(sim_mode=False):
    nc = bacc.Bacc("TRN2", target_bir_lowering=False, debug=False,
                   num_devices=1 if sim_mode else N_CORES)

    # ---- I/O ----
    zt1 = nc.dram_tensor("zt1", [128, KT, NB], F16, kind="ExternalInput").ap()
    zt2 = nc.dram_tensor("zt2", [128, KT, NB], F16, kind="ExternalInput").ap()
    w1t = nc.dram_tensor("w1t", [128, KT, D], F16, kind="ExternalInput").ap()
    w2t = nc.dram_tensor("w2t", [128, KT, D], F16, kind="ExternalInput").ap()
    b1 = nc.dram_tensor("b1", [128, KT], F32, kind="ExternalInput").ap()
    b2 = nc.dram_tensor("b2", [128, KT], F32, kind="ExternalInput").ap()
    out = nc.dram_tensor("out", [1, 1], F32, kind="ExternalOutput").ap()

    rg = [list(range(N_CORES))]

    with tile.TileContext(nc) as tc:
        with tc.tile_pool(name="persist", bufs=1) as persist, \
             tc.tile_pool(name="dram", bufs=1, space="DRAM") as dram, \
             tc.tile_pool(name="stats", bufs=1) as stats:

            ones_sc = persist.tile([1, 128], F32)
            nc.vector.memset(ones_sc[:], 1.0)
            ones_cs = persist.tile([128, 1], F32)
            nc.vector.memset(ones_cs[:], 1.0)
            ones_col = persist.tile([128, 1], F32R)
            nc.vector.tensor_copy(ones_col[:], ones_cs[:])
            ones_row = persist.tile([1, 128], F32R)
            nc.vector.tensor_copy(ones_row[:], ones_sc[:])
            ones_16 = persist.tile([128, 1], F16)
            nc.vector.memset(ones_16[:], 1.0)

            # normalized fp8 local blocks [feature, node] (sims lhsT)
            n8 = [persist.tile([128, KT, NB], F8, name=f"n8_{v}")
                  for v in range(2)]
            # 1/norm per node [1, NB]
            rn_vec = [persist.tile([1, NB], F32R, name=f"rn{v}") for v in range(2)]
            # gathered normalized embeddings, full row [feature, all nodes]
            g_sb = [persist.tile([128, KT, N], F8, name=f"g{v}") for v in range(2)]
            # colsum accumulator for exp(S12)
            acc = persist.tile([128, N], F16, name="acc")
            # fp32 projections (for the exact pos diagonal)
            h_sb = [persist.tile([128, KT, NB], F32, name=f"h{v}")
                    for v in range(2)]

            cc_in = [dram.tile([D, NB], F8, name=f"cc_in{v}") for v in range(2)]
            cc_out = [dram.tile([N_CORES * D, NB], F8, name=f"cc_out{v}",
                                addr_space="Shared",
                                tag=("agbuf0" if v == 0 else "agbuf1"))
                      for v in range(2)]
            cs_in = dram.tile([N], F32, name="cs_in")
            cs_out = dram.tile([NB], F32, name="cs_out")
            pos_part = stats.tile([1, NB], F32, name="pos_part")

            # exp row-sum partials per matrix: [128, MT, NG]
            parts = [stats.tile([128, MT, NG], F32, name=f"parts{x}")
                     for x in range(3)]  # 0=S11, 1=S12, 2=S22
            rs = [stats.tile([128, MT], F32, name=f"rs{x}") for x in range(3)]
            rs21 = stats.tile([128, MT], F32, name="rs21")
            d1g = stats.tile([128, MT], F32, name="d1g")
            pos_sum = stats.tile([1, 1], F32)

            # ---------------- projection phase ----------------
            with tc.tile_pool(name="proj", bufs=1) as proj, \
                 tc.tile_pool(name="ptmp", bufs=2) as ptmp, \
                 tc.tile_pool(name="ppsum", bufs=4, space="PSUM") as ppsum, \
                 tc.tile_pool(name="spsum", bufs=2, space="PSUM") as spsum:

                zt_sb = proj.tile([128, KT, NB], F16, name="zt_sb")
                w1_sb = proj.tile([128, KT, D], F16)
                w2_sb = proj.tile([128, KT, D], F16)
                b1_sb = proj.tile([128, KT], F32)
                b2_sb = proj.tile([128, KT], F32)
                e_sb = proj.tile([128, KT, NB], F16)
                hsq = proj.tile([128, KT, NB], F32R)

                nc.sync.dma_start(w1_sb[:], w1t)
                nc.sync.dma_start(w2_sb[:], w2t)
                nc.sync.dma_start(b1_sb[:], b1)
                nc.sync.dma_start(b2_sb[:], b2)

                for v in range(2):
                    nc.sync.dma_start(zt_sb[:], zt1 if v == 0 else zt2)
                    # ---- layer 1 + ELU ----
                    for pt in range(KT):
                        for ch in range(NB // NCHUNK):
                            ps = ppsum.tile([128, NCHUNK], F32, tag="ps_proj")
                            for kt in range(KT):
                                nc.tensor.matmul(
                                    ps[:],
                                    lhsT=w1_sb[:, kt, pt * 128:(pt + 1) * 128],
                                    rhs=zt_sb[:, kt,
                                              ch * NCHUNK:(ch + 1) * NCHUNK],
                                    start=(kt == 0), stop=(kt == KT - 1))
                            # elu(y) = max(y,0) + min(exp(y),1) - 1,  y = ps + b1
                            texp = ptmp.tile([128, NCHUNK], F16, tag="texp")
                            nc.scalar.activation(texp[:], ps[:], AF.Exp,
                                                 bias=b1_sb[:, pt:pt + 1],
                                                 scale=1.0)
                            tclip = ptmp.tile([128, NCHUNK], F16, tag="tclip")
                            nc.vector.tensor_scalar(tclip[:], texp[:], 1.0, -1.0,
                                                    ALU.min, ALU.add)
                            tmax = ptmp.tile([128, NCHUNK], F16, tag="tmax")
                            nc.scalar.activation(tmax[:], ps[:], AF.Relu,
                                                 bias=b1_sb[:, pt:pt + 1],
                                                 scale=1.0)
                            nc.vector.tensor_tensor(
                                e_sb[:, pt, ch * NCHUNK:(ch + 1) * NCHUNK],
                                tmax[:], tclip[:], ALU.add)
                    # ---- layer 2 (+ b2 on DVE), squares on DVE ----
                    for jt in range(KT):
                        for ch in range(NB // NCHUNK):
                            ps = ppsum.tile([128, NCHUNK], F32, tag="ps_proj")
                            for kt in range(KT):
                                nc.tensor.matmul(
                                    ps[:],
                                    lhsT=w2_sb[:, kt, jt * 128:(jt + 1) * 128],
                                    rhs=e_sb[:, kt, ch * NCHUNK:(ch + 1) * NCHUNK],
                                    start=(kt == 0), stop=(kt == KT - 1))
                            sl = (slice(None), jt,
                                  slice(ch * NCHUNK, (ch + 1) * NCHUNK))
                            nc.vector.tensor_scalar(h_sb[v][sl], ps[:],
                                                    b2_sb[:, jt:jt + 1], None,
                                                    ALU.add)
                            nc.scalar.activation(hsq[sl], h_sb[v][sl], AF.Square)
                    # ---- per-node 1/norm: rn = exp(-0.5*ln(ss)).  Batch the
                    # Ln's then the Exp's so the ACT table switches only twice
                    # per view instead of per chunk.
                    tlns = []
                    for ch in range(NB // NCHUNK):
                        csl = slice(ch * NCHUNK, (ch + 1) * NCHUNK)
                        pss = spsum.tile([1, NCHUNK], F32, tag="ps_small")
                        for jt in range(KT):
                            nc.tensor.matmul(
                                pss[:],
                                lhsT=ones_col[:],
                                rhs=hsq[:, jt, csl],
                                start=(jt == 0), stop=(jt == KT - 1))
                        tln = ptmp.tile([1, NCHUNK], F32, tag="tln")
                        nc.scalar.activation(tln[:], pss[:], AF.Ln)
                        tlns.append(tln)
                    for ch in range(NB // NCHUNK):
                        csl = slice(ch * NCHUNK, (ch + 1) * NCHUNK)
                        nc.scalar.activation(rn_vec[v][:, csl], tlns[ch][:],
                                             AF.Exp, scale=-0.5)
                    for ch in range(NB // NCHUNK):
                        csl = slice(ch * NCHUNK, (ch + 1) * NCHUNK)
                        # broadcast rn across partitions (K=1 ones-matmul)
                        pbc = spsum.tile([128, NCHUNK], F32, tag="ps_bc")
                        nc.tensor.matmul(
                            pbc[:], lhsT=ones_row[:],
                            rhs=rn_vec[v][:, csl],
                            start=True, stop=True)
                        for jt in range(KT):
                            nc.vector.tensor_tensor(
                                n8[v][:, jt, csl], h_sb[v][:, jt, csl],
                                pbc[:], ALU.mult)

                    # ship to DRAM + AllGather (fp8; overlaps with the other
                    # view / the sims)
                    nc.sync.dma_start(
                        cc_in[v][:].rearrange("(ko p) m -> p ko m", p=128),
                        n8[v][:])
                    if sim_mode:
                        nc.sync.dma_start(cc_out[v][0:D, :], cc_in[v][:])
                    else:
                        nc.gpsimd.collective_compute(
                            "AllGather", ALU.bypass, replica_groups=rg,
                            ins=[cc_in[v].opt()], outs=[cc_out[v].opt()])

                # ---- pos diagonal: s12_ii = rn1_i*rn2_i*sum_f h1[f,i]h2[f,i]
                hh = hsq  # reuse
                for jt in range(KT):
                    nc.vector.tensor_tensor(hh[:, jt, :], h_sb[0][:, jt, :],
                                            h_sb[1][:, jt, :], ALU.mult)
                for ch in range(NB // NCHUNK):
                    csl = slice(ch * NCHUNK, (ch + 1) * NCHUNK)
                    psp = spsum.tile([1, NCHUNK], F32, tag="ps_small")
                    for jt in range(KT):
                        nc.tensor.matmul(psp[:],
                                         lhsT=ones_col[:],
                                         rhs=hh[:, jt, csl],
                                         start=(jt == 0), stop=(jt == KT - 1))
                    nc.vector.tensor_tensor(pos_part[:, csl], psp[:],
                                            rn_vec[0][:, csl], ALU.mult)
                    nc.vector.tensor_tensor(pos_part[:, csl], pos_part[:, csl],
                                            rn_vec[1][:, csl], ALU.mult)
                nc.vector.tensor_reduce(pos_sum[:], pos_part[:],
                                        mybir.AxisListType.X, ALU.add)

            # ---------------- load gathered embeddings ----------------
            for v in range(2):
                for r in range(N_CORES):
                    nc.sync.dma_start(
                        g_sb[v][:, :, r * NB:(r + 1) * NB],
                        cc_out[v][r * D:(r + 1) * D, :]
                        .rearrange("(ko p) m -> p ko m", p=128))

            # ---------------- sims: S11 then S12 ----------------
            # fp8 DoubleRow, K=256 per step.  [128, 2048] psum groups,
            # exp(2s) + row sums on ACT; S12's exp tiles also accumulate
            # into `acc` (DVE) for the S21 row sums (colsums of exp(S12)).
            def sim_pass(x, vl, vr, pool, scr, do_acc):
                for mt in range(MT):
                    for g in range(NG):
                        pss = pool.tile([128, GW], F32, tag="ps_sim")
                        for kt2 in range(KT // 2):
                            for ch in range(GW // NCHUNK):
                                c0 = g * GW + ch * NCHUNK
                                nc.tensor.matmul(
                                    pss[:, ch * NCHUNK:(ch + 1) * NCHUNK],
                                    lhsT=n8[vl][:, 2 * kt2:2 * kt2 + 2,
                                                mt * 128:(mt + 1) * 128],
                                    rhs=g_sb[vr][:, 2 * kt2:2 * kt2 + 2,
                                                 c0:c0 + NCHUNK],
                                    start=(kt2 == 0), stop=(kt2 == KT // 2 - 1),
                                    perf_mode=DR)
                        es = scr.tile([128, GW], F16, tag="es")
                        if do_acc:
                            # S12: keep the fused ACT accum (DVE is busy with
                            # the colsum adds here)
                            nc.scalar.activation(
                                es[:], pss[:], AF.Exp, scale=TAU_INV,
                                accum_out=parts[x][:, mt, g:g + 1])
                            asl = acc[:, g * GW:(g + 1) * GW]
                            if mt == 0:
                                nc.vector.tensor_copy(asl, es[:])
                            else:
                                nc.vector.tensor_tensor(asl, asl, es[:],
                                                        ALU.add)
                        else:
                            # S11/S22: row sums on the otherwise-idle DVE,
                            # saving the 283ns ACTIVATION_READ_ACCUMULATOR
                            # per group on the bottleneck scalar engine
                            nc.scalar.activation(es[:], pss[:], AF.Exp,
                                                 scale=TAU_INV)
                            nc.vector.tensor_reduce(
                                parts[x][:, mt, g:g + 1], es[:],
                                mybir.AxisListType.X, ALU.add)

            with tc.tile_pool(name="sim_psum", bufs=2, space="PSUM") as sp, \
                 tc.tile_pool(name="scr", bufs=4) as scr, \
                 tc.tile_pool(name="cs_sbp", bufs=4) as cs_sbp:
                sim_pass(0, 0, 0, sp, scr, False)   # S11
                sim_pass(1, 0, 1, sp, scr, True)    # S12 (+ colsum acc)

                # d1 = ln(rs11 + rs12 - e^2) can complete during S22
                nc.vector.tensor_reduce(rs[0][:], parts[0][:],
                                        mybir.AxisListType.X, ALU.add)
                nc.vector.tensor_reduce(rs[1][:], parts[1][:],
                                        mybir.AxisListType.X, ALU.add)
                nc.vector.tensor_tensor(d1g[:], rs[0][:], rs[1][:], ALU.add)
                nc.vector.tensor_scalar_add(d1g[:], d1g[:], -E2)
                nc.scalar.activation(d1g[:], d1g[:], AF.Ln)

                # ---- S22, with the S21 colsum reduction interleaved.
                # After row-tiles 2..5 of S22, borrow one sim-psum ring slot
                # for 4 ones-matmul partition reductions of `acc`; DVE copies
                # them out and small DMAs stream them to cs_in.  The
                # ReduceScatter then overlaps the tail of S22.
                for mt in range(MT):
                    for g in range(NG):
                        pss = sp.tile([128, GW], F32, tag="ps_sim")
                        for kt2 in range(KT // 2):
                            for ch in range(GW // NCHUNK):
                                c0 = g * GW + ch * NCHUNK
                                nc.tensor.matmul(
                                    pss[:, ch * NCHUNK:(ch + 1) * NCHUNK],
                                    lhsT=n8[1][:, 2 * kt2:2 * kt2 + 2,
                                               mt * 128:(mt + 1) * 128],
                                    rhs=g_sb[1][:, 2 * kt2:2 * kt2 + 2,
                                                c0:c0 + NCHUNK],
                                    start=(kt2 == 0), stop=(kt2 == KT // 2 - 1),
                                    perf_mode=DR)
                        es = scr.tile([128, GW], F16, tag="es")
                        nc.scalar.activation(es[:], pss[:], AF.Exp,
                                             scale=TAU_INV)
                        nc.vector.tensor_reduce(
                            parts[2][:, mt, g:g + 1], es[:],
                            mybir.AxisListType.X, ALU.add)
                    if mt <= 3:
                        rnd = mt
                        pcol = sp.tile([128, GW], F32, tag="ps_sim")
                        for i in range(4):
                            c = rnd * 4 + i
                            nc.tensor.matmul(
                                pcol[0:1, i * NCHUNK:(i + 1) * NCHUNK],
                                lhsT=ones_16[:],
                                rhs=acc[:, c * NCHUNK:(c + 1) * NCHUNK],
                                start=True, stop=True)
                        for i in range(4):
                            c = rnd * 4 + i
                            cst = cs_sbp.tile([1, NCHUNK], F32, tag="cs")
                            nc.vector.tensor_copy(
                                cst[:], pcol[0:1, i * NCHUNK:(i + 1) * NCHUNK])
                            nc.sync.dma_start(
                                cs_in[c * NCHUNK:(c + 1) * NCHUNK], cst[:])
                    if mt == 3:
                        if sim_mode:
                            nc.sync.dma_start(cs_out[:], cs_in[0:NB])
                        else:
                            nc.gpsimd.collective_compute(
                                "ReduceScatter", ALU.add, replica_groups=rg,
                                ins=[cs_in.opt()], outs=[cs_out.opt()])
                        nc.sync.dma_start(
                            rs21[:],
                            cs_out.rearrange("(mt p) -> p mt", p=128))

            # ---------------- assemble the loss ----------------
            with tc.tile_pool(name="fin", bufs=1) as fsb, \
                 tc.tile_pool(name="fin_psum", bufs=1, space="PSUM") as fp:
                nc.vector.tensor_reduce(rs[2][:], parts[2][:],
                                        mybir.AxisListType.X, ALU.add)
                d2 = fsb.tile([128, MT], F32)
                nc.vector.tensor_tensor(d2[:], rs[2][:], rs21[:], ALU.add)
                nc.vector.tensor_scalar_add(d2[:], d2[:], -E2)
                nc.scalar.activation(d2[:], d2[:], AF.Ln)
                lsum = fsb.tile([128, MT], F32)
                nc.vector.tensor_tensor(lsum[:], d1g[:], d2[:], ALU.add)
                lrow = fsb.tile([128, 1], F32)
                nc.vector.tensor_reduce(lrow[:], lsum[:],
                                        mybir.AxisListType.X, ALU.add)
                pfin = fp.tile([1, 1], F32)
                nc.tensor.matmul(pfin[:], lhsT=ones_cs[:], rhs=lrow[:],
                                 start=True, stop=True)
                fin = fsb.tile([1, 1], F32)
                nc.vector.tensor_scalar_mul(fin[:], pfin[:], 0.5)
                p2 = fsb.tile([1, 1], F32)
                nc.vector.tensor_scalar_mul(p2[:], pos_sum[:], 2.0)
                nc.vector.tensor_tensor(fin[:], fin[:], p2[:], ALU.subtract)
                nc.sync.dma_start(out, fin[:])

    nc.compile()
    return nc


def _prep_inputs(z1, z2, fc1_w, fc1_b, fc2_w, fc2_b):
    """Host-side shard + layout prep. Returns in_maps for the 8 cores."""
    w1t = np.ascontiguousarray(fc1_w.T).reshape(KT, 128, D).transpose(1, 0, 2)
    w1t = np.ascontiguousarray(w1t, dtype=np.float16)
    w2t = np.ascontiguousarray(fc2_w.T).reshape(KT, 128, D).transpose(1, 0, 2)
    w2t = np.ascontiguousarray(w2t, dtype=np.float16)
    b1 = np.ascontiguousarray(fc1_b.reshape(KT, 128).T, dtype=np.float32)
    b2 = np.ascontiguousarray(fc2_b.reshape(KT, 128).T, dtype=np.float32)

    in_maps = []
    for c in range(N_CORES):
        blk1 = z1[c * NB:(c + 1) * NB].T            # [512, 1024]
        blk2 = z2[c * NB:(c + 1) * NB].T
        zt1 = np.ascontiguousarray(
            blk1.reshape(KT, 128, NB).transpose(1, 0, 2), dtype=np.float16)
        zt2 = np.ascontiguousarray(
            blk2.reshape(KT, 128, NB).transpose(1, 0, 2), dtype=np.float16)
        in_maps.append({"zt1": zt1, "zt2": zt2, "w1t": w1t, "w2t": w2t,
                        "b1": b1, "b2": b2})
    return in_maps


def kernel(z1, z2, fc1_w, fc1_b, fc2_w, fc2_b):
    global LAST_EXEC_NS
    z1 = np.asarray(z1, dtype=np.float32)
    z2 = np.asarray(z2, dtype=np.float32)
    fc1_w = np.asarray(fc1_w, dtype=np.float32)
    fc1_b = np.asarray(fc1_b, dtype=np.float32)
    fc2_w = np.asarray(fc2_w, dtype=np.float32)
    fc2_b = np.asarray(fc2_b, dtype=np.float32)

    if "nc" not in _CACHE:
        _CACHE["nc"] = _build_program()
    nc = _CACHE["nc"]

    in_maps = _prep_inputs(z1, z2, fc1_w, fc1_b, fc2_w, fc2_b)
    res = run_bass_kernel_spmd(nc, in_maps, core_ids=list(range(N_CORES)),
                               trace=TRACE)
    LAST_EXEC_NS = res.exec_time_ns
    total = math.fsum(float(r["out"][0, 0]) for r in res.results)
    return np.float32(total / N)


# revision 22
# speedup vs baseline: 1.1025x; 1.1025x over previous
"""GRACE contrastive loss kernel for Trainium2 (8 NeuronCores, SPMD).

Strategy (row-block data parallel):
  - Shard the N=8192 nodes across 8 cores (1024 rows each).
  - Each core projects its z1/z2 block through the 2-layer MLP (fp16 matmuls,
    fp32 accum), computes per-node 1/norms as exp(-0.5*ln(sum h^2)) on the
    activation engine (avoids slow 1-lane reciprocals; Ln/Exp batched so the
    ACT table switches only twice per view), quantizes the normalized
    embeddings to fp8e4, and AllGathers them (fp8, 4.2 MB/view) so every
    core holds full gathered n1/n2 [512, 8192] in SBUF.
  - Similarities run as fp8 DoubleRow matmuls (K=256 per step, 2 steps) in
    [128 x 2048] PSUM groups (double-buffered, 8 banks) with fused exp(2*s)
    + row-sum on the scalar engine (accum_out).  Steady state is jointly
    scalar/PE bound at ~2.2us per 2048-column group.
  - Only S11, S12, S22 are computed explicitly; S21's row sums (= column
    sums of exp(S12) over the full matrix) come from a DVE fp16 2x
    accumulation of the exp(S12) tiles into a [128, 8192] buffer, fp16
    ones-matmul partition reductions interleaved into the first four S22
    row-tiles (borrowing a sim-psum slot), and a ReduceScatter(add) that
    overlaps the back half of S22.
  - The positive diagonal s12_ii is computed exactly in fp32 from h1/h2.
  - Per-core scalar partial out; host sums partials / N.

Measured: 352.9 us HW exec (baseline 718.9 us), rel err 1.2e-05.
"""

import math
import sys

import numpy as np

sys.path.insert(0, "/opt/trn_rl_repo")

import concourse.bass as bass  # noqa: E402
import concourse.mybir as mybir  # noqa: E402
import concourse.tile as tile  # noqa: E402
from concourse import bacc  # noqa: E402
from concourse.bass_utils import run_bass_kernel_spmd  # noqa: E402

F32 = mybir.dt.float32
F32R = mybir.dt.float32r
F16 = mybir.dt.float16
F8 = mybir.dt.float8e4
AF = mybir.ActivationFunctionType
ALU = mybir.AluOpType
DR = mybir.MatmulPerfMode.DoubleRow

N_CORES = 8
N = 8192
D = 512            # feature dim (= H = P in the reference MLP)
NB = N // N_CORES  # 1024 rows per core
KT = D // 128      # 4 k-subtiles
MT = NB // 128     # 8 row tiles per core
NCHUNK = 512
GW = 2048          # sim column-group width (4 psum banks)
NG = N // GW       # 4 groups per row tile
TAU_INV = 2.0      # 1 / tau
E2 = float(np.exp(2.0, dtype=np.float64))  # exp(diag(refl_sim)/tau), diag == 1

TRACE = False
LAST_EXEC_NS = None
_CACHE = {}


def _build_program(sim_mode=False):
    nc = bacc.Bacc("TRN2", target_bir_lowering=False, debug=False,
                   num_devices=1 if sim_mode else N_CORES)

    # ---- I/O ----
    zt1 = nc.dram_tensor("zt1", [128, KT, NB], F16, kind="ExternalInput").ap()
    zt2 = nc.dram_tensor("zt2", [128, KT, NB], F16, kind="ExternalInput").ap()
    w1t = nc.dram_tensor("w1t", [128, KT, D], F16, kind="ExternalInput").ap()
    w2t = nc.dram_tensor("w2t", [128, KT, D], F16, kind="ExternalInput").ap()
    b1 = nc.dram_tensor("b1", [128, KT], F32, kind="ExternalInput").ap()
    b2 = nc.dram_tensor("b2", [128, KT], F32, kind="ExternalInput").ap()
    out = nc.dram_tensor("out", [1, 1], F32, kind="ExternalOutput").ap()

    rg = [list(range(N_CORES))]

    with tile.TileContext(nc) as tc:
        with tc.tile_pool(name="persist", bufs=1) as persist, \
             tc.tile_pool(name="dram", bufs=1, space="DRAM") as dram, \
             tc.tile_pool(name="stats", bufs=1) as stats:

            ones_sc = persist.tile([1, 128], F32)
            nc.vector.memset(ones_sc[:], 1.0)
            ones_cs = persist.tile([128, 1], F32)
            nc.vector.memset(ones_cs[:], 1.0)
            ones_col = persist.tile([128, 1], F32R)
            nc.vector.tensor_copy(ones_col[:], ones_cs[:])
            ones_row = persist.tile([1, 128], F32R)
            nc.vector.tensor_copy(ones_row[:], ones_sc[:])
            ones_16 = persist.tile([128, 1], F16)
            nc.vector.memset(ones_16[:], 1.0)

            # normalized fp8 local blocks [feature, node] (sims lhsT)
            n8 = [persist.tile([128, KT, NB], F8, name=f"n8_{v}")
                  for v in range(2)]
            # 1/norm per node [1, NB]
            rn_vec = [persist.tile([1, NB], F32R, name=f"rn{v}") for v in range(2)]
            # gathered normalized embeddings, full row [feature, all nodes]
            g_sb = [persist.tile([128, KT, N], F8, name=f"g{v}") for v in range(2)]
            # colsum accumulator for exp(S12)
            acc = persist.tile([128, N], F16, name="acc")
            # fp32 projections (for the exact pos diagonal)
            h_sb = [persist.tile([128, KT, NB], F32, name=f"h{v}")
                    for v in range(2)]

            cc_in = [dram.tile([D, NB], F8, name=f"cc_in{v}") for v in range(2)]
            cc_out = [dram.tile([N_CORES * D, NB], F8, name=f"cc_out{v}",
                                addr_space="Shared",
                                tag=("agbuf0" if v == 0 else "agbuf1"))
                      for v in range(2)]
            cs_in = dram.tile([N], F32, name="cs_in")
            cs_out = dram.tile([NB], F32, name="cs_out")
            pos_part = stats.tile([1, NB], F32, name="pos_part")

            # exp row-sum partials per matrix: [128, MT, NG]
            parts = [stats.tile([128, MT, NG], F32, name=f"parts{x}")
                     for x in range(3)]  # 0=S11, 1=S12, 2=S22
            rs = [stats.tile([128, MT], F32, name=f"rs{x}") for x in range(3)]
            rs21 = stats.tile([128, MT], F32, name="rs21")
            pos_sum = stats.tile([1, 1], F32)

            # ---------------- projection phase ----------------
            with tc.tile_pool(name="proj", bufs=1) as proj, \
                 tc.tile_pool(name="ptmp", bufs=2) as ptmp, \
                 tc.tile_pool(name="ppsum", bufs=4, space="PSUM") as ppsum, \
                 tc.tile_pool(name="spsum", bufs=2, space="PSUM") as spsum:

                zt_sb = proj.tile([128, KT, NB], F16, name="zt_sb")
                w1_sb = proj.tile([128, KT, D], F16)
                w2_sb = proj.tile([128, KT, D], F16)
                b1_sb = proj.tile([128, KT], F32)
                b2_sb = proj.tile([128, KT], F32)
                e_sb = proj.tile([128, KT, NB], F16)
                hsq = proj.tile([128, KT, NB], F32R)

                nc.sync.dma_start(w1_sb[:], w1t)
                nc.sync.dma_start(w2_sb[:], w2t)
                nc.sync.dma_start(b1_sb[:], b1)
                nc.sync.dma_start(b2_sb[:], b2)

                for v in range(2):
                    nc.sync.dma_start(zt_sb[:], zt1 if v == 0 else zt2)
                    # ---- layer 1 + ELU ----
                    for pt in range(KT):
                        for ch in range(NB // NCHUNK):
                            ps = ppsum.tile([128, NCHUNK], F32, tag="ps_proj")
                            for kt in range(KT):
                                nc.tensor.matmul(
                                    ps[:],
                                    lhsT=w1_sb[:, kt, pt * 128:(pt + 1) * 128],
                                    rhs=zt_sb[:, kt,
                                              ch * NCHUNK:(ch + 1) * NCHUNK],
                                    start=(kt == 0), stop=(kt == KT - 1))
                            # elu(y) = max(y,0) + min(exp(y),1) - 1,  y = ps + b1
                            texp = ptmp.tile([128, NCHUNK], F16, tag="texp")
                            nc.scalar.activation(texp[:], ps[:], AF.Exp,
                                                 bias=b1_sb[:, pt:pt + 1],
                                                 scale=1.0)
                            tclip = ptmp.tile([128, NCHUNK], F16, tag="tclip")
                            nc.vector.tensor_scalar(tclip[:], texp[:], 1.0, -1.0,
                                                    ALU.min, ALU.add)
                            tmax = ptmp.tile([128, NCHUNK], F16, tag="tmax")
                            nc.scalar.activation(tmax[:], ps[:], AF.Relu,
                                                 bias=b1_sb[:, pt:pt + 1],
                                                 scale=1.0)
                            nc.vector.tensor_tensor(
                                e_sb[:, pt, ch * NCHUNK:(ch + 1) * NCHUNK],
                                tmax[:], tclip[:], ALU.add)
                    # ---- layer 2 (+ b2 on DVE), squares on DVE ----
                    for jt in range(KT):
                        for ch in range(NB // NCHUNK):
                            ps = ppsum.tile([128, NCHUNK], F32, tag="ps_proj")
                            for kt in range(KT):
                                nc.tensor.matmul(
                                    ps[:],
                                    lhsT=w2_sb[:, kt, jt * 128:(jt + 1) * 128],
                                    rhs=e_sb[:, kt, ch * NCHUNK:(ch + 1) * NCHUNK],
                                    start=(kt == 0), stop=(kt == KT - 1))
                            sl = (slice(None), jt,
                                  slice(ch * NCHUNK, (ch + 1) * NCHUNK))
                            nc.vector.tensor_scalar(h_sb[v][sl], ps[:],
                                                    b2_sb[:, jt:jt + 1], None,
                                                    ALU.add)
                            nc.scalar.activation(hsq[sl], h_sb[v][sl], AF.Square)
                    # ---- per-node 1/norm: rn = exp(-0.5*ln(ss)).  Batch the
                    # Ln's then the Exp's so the ACT table switches only twice
                    # per view instead of per chunk.
                    tlns = []
                    for ch in range(NB // NCHUNK):
                        csl = slice(ch * NCHUNK, (ch + 1) * NCHUNK)
                        pss = spsum.tile([1, NCHUNK], F32, tag="ps_small")
                        for jt in range(KT):
                            nc.tensor.matmul(
                                pss[:],
                                lhsT=ones_col[:],
                                rhs=hsq[:, jt, csl],
                                start=(jt == 0), stop=(jt == KT - 1))
                        tln = ptmp.tile([1, NCHUNK], F32, tag="tln")
                        nc.scalar.activation(tln[:], pss[:], AF.Ln)
                        tlns.append(tln)
                    for ch in range(NB // NCHUNK):
                        csl = slice(ch * NCHUNK, (ch + 1) * NCHUNK)
                        nc.scalar.activation(rn_vec[v][:, csl], tlns[ch][:],
                                             AF.Exp, scale=-0.5)
                    for ch in range(NB // NCHUNK):
                        csl = slice(ch * NCHUNK, (ch + 1) * NCHUNK)
                        # broadcast rn across partitions (K=1 ones-matmul)
                        pbc = spsum.tile([128, NCHUNK], F32, tag="ps_bc")
                        nc.tensor.matmul(
                            pbc[:], lhsT=ones_row[:],
                            rhs=rn_vec[v][:, csl],
                            start=True, stop=True)
                        for jt in range(KT):
                            nc.vector.tensor_tensor(
                                n8[v][:, jt, csl], h_sb[v][:, jt, csl],
                                pbc[:], ALU.mult)

                    # ship to DRAM + AllGather (fp8; overlaps with the other
                    # view / the sims)
                    nc.sync.dma_start(
                        cc_in[v][:].rearrange("(ko p) m -> p ko m", p=128),
                        n8[v][:])
                    if sim_mode:
                        nc.sync.dma_start(cc_out[v][0:D, :], cc_in[v][:])
                    else:
                        nc.gpsimd.collective_compute(
                            "AllGather", ALU.bypass, replica_groups=rg,
                            ins=[cc_in[v].opt()], outs=[cc_out[v].opt()])

                # ---- pos diagonal: s12_ii = rn1_i*rn2_i*sum_f h1[f,i]h2[f,i]
                hh = hsq  # reuse
                for jt in range(KT):
                    nc.vector.tensor_tensor(hh[:, jt, :], h_sb[0][:, jt, :],
                                            h_sb[1][:, jt, :], ALU.mult)
                for ch in range(NB // NCHUNK):
                    csl = slice(ch * NCHUNK, (ch + 1) * NCHUNK)
                    psp = spsum.tile([1, NCHUNK], F32, tag="ps_small")
                    for jt in range(KT):
                        nc.tensor.matmul(psp[:],
                                         lhsT=ones_col[:],
                                         rhs=hh[:, jt, csl],
                                         start=(jt == 0), stop=(jt == KT - 1))
                    nc.vector.tensor_tensor(pos_part[:, csl], psp[:],
                                            rn_vec[0][:, csl], ALU.mult)
                    nc.vector.tensor_tensor(pos_part[:, csl], pos_part[:, csl],
                                            rn_vec[1][:, csl], ALU.mult)
                nc.vector.tensor_reduce(pos_sum[:], pos_part[:],
                                        mybir.AxisListType.X, ALU.add)

            # ---------------- load gathered embeddings ----------------
            for v in range(2):
                for r in range(N_CORES):
                    nc.sync.dma_start(
                        g_sb[v][:, :, r * NB:(r + 1) * NB],
                        cc_out[v][r * D:(r + 1) * D, :]
                        .rearrange("(ko p) m -> p ko m", p=128))

            # ---------------- sims: S11 then S12 ----------------
            # fp8 DoubleRow, K=256 per step.  [128, 2048] psum groups,
            # exp(2s) + row sums on ACT; S12's exp tiles also accumulate
            # into `acc` (DVE) for the S21 row sums (colsums of exp(S12)).
            def sim_pass(x, vl, vr, pool, scr, do_acc):
                for mt in range(MT):
                    for g in range(NG):
                        pss = pool.tile([128, GW], F32, tag="ps_sim")
                        for kt2 in range(KT // 2):
                            for ch in range(GW // NCHUNK):
                                c0 = g * GW + ch * NCHUNK
                                nc.tensor.matmul(
                                    pss[:, ch * NCHUNK:(ch + 1) * NCHUNK],
                                    lhsT=n8[vl][:, 2 * kt2:2 * kt2 + 2,
                                                mt * 128:(mt + 1) * 128],
                                    rhs=g_sb[vr][:, 2 * kt2:2 * kt2 + 2,
                                                 c0:c0 + NCHUNK],
                                    start=(kt2 == 0), stop=(kt2 == KT // 2 - 1),
                                    perf_mode=DR)
                        es = scr.tile([128, GW], F16, tag="es")
                        nc.scalar.activation(
                            es[:], pss[:], AF.Exp, scale=TAU_INV,
                            accum_out=parts[x][:, mt, g:g + 1])
                        if do_acc:
                            asl = acc[:, g * GW:(g + 1) * GW]
                            if mt == 0:
                                nc.vector.tensor_copy(asl, es[:])
                            else:
                                nc.vector.tensor_tensor(asl, asl, es[:],
                                                        ALU.add)

            with tc.tile_pool(name="sim_psum", bufs=2, space="PSUM") as sp, \
                 tc.tile_pool(name="scr", bufs=4) as scr, \
                 tc.tile_pool(name="cs_sbp", bufs=4) as cs_sbp:
                sim_pass(0, 0, 0, sp, scr, False)   # S11
                sim_pass(1, 0, 1, sp, scr, True)    # S12 (+ colsum acc)

                # ---- S22, with the S21 colsum reduction interleaved.
                # After row-tiles 2..5 of S22, borrow one sim-psum ring slot
                # for 4 ones-matmul partition reductions of `acc`; DVE copies
                # them out and small DMAs stream them to cs_in.  The
                # ReduceScatter then overlaps the tail of S22.
                for mt in range(MT):
                    for g in range(NG):
                        pss = sp.tile([128, GW], F32, tag="ps_sim")
                        for kt2 in range(KT // 2):
                            for ch in range(GW // NCHUNK):
                                c0 = g * GW + ch * NCHUNK
                                nc.tensor.matmul(
                                    pss[:, ch * NCHUNK:(ch + 1) * NCHUNK],
                                    lhsT=n8[1][:, 2 * kt2:2 * kt2 + 2,
                                               mt * 128:(mt + 1) * 128],
                                    rhs=g_sb[1][:, 2 * kt2:2 * kt2 + 2,
                                                c0:c0 + NCHUNK],
                                    start=(kt2 == 0), stop=(kt2 == KT // 2 - 1),
                                    perf_mode=DR)
                        es = scr.tile([128, GW], F16, tag="es")
                        nc.scalar.activation(
                            es[:], pss[:], AF.Exp, scale=TAU_INV,
                            accum_out=parts[2][:, mt, g:g + 1])
                    if mt <= 3:
                        rnd = mt
                        pcol = sp.tile([128, GW], F32, tag="ps_sim")
                        for i in range(4):
                            c = rnd * 4 + i
                            nc.tensor.matmul(
                                pcol[0:1, i * NCHUNK:(i + 1) * NCHUNK],
                                lhsT=ones_16[:],
                                rhs=acc[:, c * NCHUNK:(c + 1) * NCHUNK],
                                start=True, stop=True)
                        for i in range(4):
                            c = rnd * 4 + i
                            cst = cs_sbp.tile([1, NCHUNK], F32, tag="cs")
                            nc.vector.tensor_copy(
                                cst[:], pcol[0:1, i * NCHUNK:(i + 1) * NCHUNK])
                            nc.sync.dma_start(
                                cs_in[c * NCHUNK:(c + 1) * NCHUNK], cst[:])
                    if mt == 3:
                        if sim_mode:
                            nc.sync.dma_start(cs_out[:], cs_in[0:NB])
                        else:
                            nc.gpsimd.collective_compute(
                                "ReduceScatter", ALU.add, replica_groups=rg,
                                ins=[cs_in.opt()], outs=[cs_out.opt()])
                        nc.sync.dma_start(
                            rs21[:],
                            cs_out.rearrange("(mt p) -> p mt", p=128))

            # ---------------- assemble the loss ----------------
            with tc.tile_pool(name="fin", bufs=1) as fsb, \
                 tc.tile_pool(name="fin_psum", bufs=1, space="PSUM") as fp:
                for x in range(3):
                    nc.vector.tensor_reduce(rs[x][:], parts[x][:],
                                            mybir.AxisListType.X, ALU.add)
                d1 = fsb.tile([128, MT], F32)
                d2 = fsb.tile([128, MT], F32)
                nc.vector.tensor_tensor(d1[:], rs[0][:], rs[1][:], ALU.add)
                nc.vector.tensor_scalar_add(d1[:], d1[:], -E2)
                nc.vector.tensor_tensor(d2[:], rs[2][:], rs21[:], ALU.add)
                nc.vector.tensor_scalar_add(d2[:], d2[:], -E2)
                nc.scalar.activation(d1[:], d1[:], AF.Ln)
                nc.scalar.activation(d2[:], d2[:], AF.Ln)
                lsum = fsb.tile([128, MT], F32)
                nc.vector.tensor_tensor(lsum[:], d1[:], d2[:], ALU.add)
                lrow = fsb.tile([128, 1], F32)
                nc.vector.tensor_reduce(lrow[:], lsum[:],
                                        mybir.AxisListType.X, ALU.add)
                pfin = fp.tile([1, 1], F32)
                nc.tensor.matmul(pfin[:], lhsT=ones_cs[:], rhs=lrow[:],
                                 start=True, stop=True)
                fin = fsb.tile([1, 1], F32)
                nc.vector.tensor_scalar_mul(fin[:], pfin[:], 0.5)
                p2 = fsb.tile([1, 1], F32)
                nc.vector.tensor_scalar_mul(p2[:], pos_sum[:], 2.0)
                nc.vector.tensor_tensor(fin[:], fin[:], p2[:], ALU.subtract)
                nc.sync.dma_start(out, fin[:])

    nc.compile()
    return nc


def _prep_inputs(z1, z2, fc1_w, fc1_b, fc2_w, fc2_b):
    """Host-side shard + layout prep. Returns in_maps for the 8 cores."""
    w1t = np.ascontiguousarray(fc1_w.T).reshape(KT, 128, D).transpose(1, 0, 2)
    w1t = np.ascontiguousarray(w1t, dtype=np.float16)
    w2t = np.ascontiguousarray(fc2_w.T).reshape(KT, 128, D).transpose(1, 0, 2)
    w2t = np.ascontiguousarray(w2t, dtype=np.float16)
    b1 = np.ascontiguousarray(fc1_b.reshape(KT, 128).T, dtype=np.float32)
    b2 = np.ascontiguousarray(fc2_b.reshape(KT, 128).T, dtype=np.float32)

    in_maps = []
    for c in range(N_CORES):
        blk1 = z1[c * NB:(c + 1) * NB].T            # [512, 1024]
        blk2 = z2[c * NB:(c + 1) * NB].T
        zt1 = np.ascontiguousarray(
            blk1.reshape(KT, 128, NB).transpose(1, 0, 2), dtype=np.float16)
        zt2 = np.ascontiguousarray(
            blk2.reshape(KT, 128, NB).transpose(1, 0, 2), dtype=np.float16)
        in_maps.append({"zt1": zt1, "zt2": zt2, "w1t": w1t, "w2t": w2t,
                        "b1": b1, "b2": b2})
    return in_maps


def kernel(z1, z2, fc1_w, fc1_b, fc2_w, fc2_b):
    global LAST_EXEC_NS
    z1 = np.asarray(z1, dtype=np.float32)
    z2 = np.asarray(z2, dtype=np.float32)
    fc1_w = np.asarray(fc1_w, dtype=np.float32)
    fc1_b = np.asarray(fc1_b, dtype=np.float32)
    fc2_w = np.asarray(fc2_w, dtype=np.float32)
    fc2_b = np.asarray(fc2_b, dtype=np.float32)

    if "nc" not in _CACHE:
        _CACHE["nc"] = _build_program()
    nc = _CACHE["nc"]

    in_maps = _prep_inputs(z1, z2, fc1_w, fc1_b, fc2_w, fc2_b)
    res = run_bass_kernel_spmd(nc, in_maps, core_ids=list(range(N_CORES)),
                               trace=TRACE)
    LAST_EXEC_NS = res.exec_time_ns
    total = math.fsum(float(r["out"][0, 0]) for r in res.results)
    return np.float32(total / N)


# revision 23
# speedup vs baseline: 1.1031x; 1.0005x over previous
"""GRACE contrastive loss kernel for Trainium2 (8 NeuronCores, SPMD).

Strategy (row-block data parallel):
  - Shard the N=8192 nodes across 8 cores (1024 rows each).
  - Each core projects its z1/z2 block through the 2-layer MLP (fp16 matmuls,
    fp32 accum), computes per-node 1/norms as exp(-0.5*ln(sum h^2)) on the
    activation engine (avoids slow 1-lane reciprocals; Ln/Exp batched so the
    ACT table switches only twice per view), quantizes the normalized
    embeddings to fp8e4, and AllGathers them (fp8, 4.2 MB/view) so every
    core holds full gathered n1/n2 [512, 8192] in SBUF.
  - Similarities run as fp8 DoubleRow matmuls (K=256 per step, 2 steps) in
    [128 x 2048] PSUM groups (double-buffered, 8 banks) with fused exp(2*s)
    + row-sum on the scalar engine (accum_out).  Steady state is jointly
    scalar/PE bound at ~2.2us per 2048-column group.
  - Only S11, S12, S22 are computed explicitly; S21's row sums (= column
    sums of exp(S12) over the full matrix) come from a DVE fp16 2x
    accumulation of the exp(S12) tiles into a [128, 8192] buffer, fp16
    ones-matmul partition reductions interleaved into the first four S22
    row-tiles (borrowing a sim-psum slot), and a ReduceScatter(add) that
    overlaps the back half of S22.
  - The positive diagonal s12_ii is computed exactly in fp32 from h1/h2.
  - Per-core scalar partial out; host sums partials / N.

Measured: 352.9 us HW exec (baseline 718.9 us), rel err 1.2e-05.
"""

import math
import sys

import numpy as np

sys.path.insert(0, "/opt/trn_rl_repo")

import concourse.bass as bass  # noqa: E402
import concourse.mybir as mybir  # noqa: E402
import concourse.tile as tile  # noqa: E402
from concourse import bacc  # noqa: E402
from concourse.bass_utils import run_bass_kernel_spmd  # noqa: E402

F32 = mybir.dt.float32
F32R = mybir.dt.float32r
F16 = mybir.dt.float16
F8 = mybir.dt.float8e4
AF = mybir.ActivationFunctionType
ALU = mybir.AluOpType
DR = mybir.MatmulPerfMode.DoubleRow

N_CORES = 8
N = 8192
D = 512            # feature dim (= H = P in the reference MLP)
NB = N // N_CORES  # 1024 rows per core
KT = D // 128      # 4 k-subtiles
MT = NB // 128     # 8 row tiles per core
NCHUNK = 512
GW = 2048          # sim column-group width (4 psum banks)
NG = N // GW       # 4 groups per row tile
TAU_INV = 2.0      # 1 / tau
E2 = float(np.exp(2.0, dtype=np.float64))  # exp(diag(refl_sim)/tau), diag == 1

TRACE = False
LAST_EXEC_NS = None
_CACHE = {}


def _build_program(sim_mode=False):
    nc = bacc.Bacc("TRN2", target_bir_lowering=False, debug=False,
                   num_devices=1 if sim_mode else N_CORES)

    # ---- I/O ----
    zt1 = nc.dram_tensor("zt1", [128, KT, NB], F16, kind="ExternalInput").ap()
    zt2 = nc.dram_tensor("zt2", [128, KT, NB], F16, kind="ExternalInput").ap()
    w1t = nc.dram_tensor("w1t", [128, KT, D], F16, kind="ExternalInput").ap()
    w2t = nc.dram_tensor("w2t", [128, KT, D], F16, kind="ExternalInput").ap()
    b1 = nc.dram_tensor("b1", [128, KT], F32, kind="ExternalInput").ap()
    b2 = nc.dram_tensor("b2", [128, KT], F32, kind="ExternalInput").ap()
    out = nc.dram_tensor("out", [1, 1], F32, kind="ExternalOutput").ap()

    rg = [list(range(N_CORES))]

    with tile.TileContext(nc) as tc:
        with tc.tile_pool(name="persist", bufs=1) as persist, \
             tc.tile_pool(name="dram", bufs=1, space="DRAM") as dram, \
             tc.tile_pool(name="stats", bufs=1) as stats:

            ones_sc = persist.tile([1, 128], F32)
            nc.vector.memset(ones_sc[:], 1.0)
            ones_cs = persist.tile([128, 1], F32)
            nc.vector.memset(ones_cs[:], 1.0)
            ones_col = persist.tile([128, 1], F32R)
            nc.vector.tensor_copy(ones_col[:], ones_cs[:])
            ones_row = persist.tile([1, 128], F32R)
            nc.vector.tensor_copy(ones_row[:], ones_sc[:])
            ones_16 = persist.tile([128, 1], F16)
            nc.vector.memset(ones_16[:], 1.0)

            # normalized fp8 local blocks [feature, node] (sims lhsT)
            n8 = [persist.tile([128, KT, NB], F8, name=f"n8_{v}")
                  for v in range(2)]
            # 1/norm per node [1, NB]
            rn_vec = [persist.tile([1, NB], F32R, name=f"rn{v}") for v in range(2)]
            # gathered normalized embeddings, full row [feature, all nodes]
            g_sb = [persist.tile([128, KT, N], F8, name=f"g{v}") for v in range(2)]
            # colsum accumulator for exp(S12)
            acc = persist.tile([128, N], F16, name="acc")
            # fp32 projections (for the exact pos diagonal)
            h_sb = [persist.tile([128, KT, NB], F32, name=f"h{v}")
                    for v in range(2)]

            cc_in = [dram.tile([D, NB], F8, name=f"cc_in{v}") for v in range(2)]
            cc_out = [dram.tile([N_CORES * D, NB], F8, name=f"cc_out{v}",
                                addr_space="Shared",
                                tag=("agbuf0" if v == 0 else "agbuf1"))
                      for v in range(2)]
            cs_in = dram.tile([N], F32, name="cs_in")
            cs_out = dram.tile([NB], F32, name="cs_out")
            pos_part = stats.tile([1, NB], F32, name="pos_part")

            # exp row-sum partials per matrix: [128, MT, NG]
            parts = [stats.tile([128, MT, NG], F32, name=f"parts{x}")
                     for x in range(3)]  # 0=S11, 1=S12, 2=S22
            rs = [stats.tile([128, MT], F32, name=f"rs{x}") for x in range(3)]
            rs21 = stats.tile([128, MT], F32, name="rs21")
            pos_sum = stats.tile([1, 1], F32)

            # ---------------- projection phase ----------------
            with tc.tile_pool(name="proj", bufs=1) as proj, \
                 tc.tile_pool(name="ptmp", bufs=2) as ptmp, \
                 tc.tile_pool(name="ppsum", bufs=4, space="PSUM") as ppsum, \
                 tc.tile_pool(name="spsum", bufs=2, space="PSUM") as spsum:

                zt_sb = proj.tile([128, KT, NB], F16, name="zt_sb")
                w1_sb = proj.tile([128, KT, D], F16)
                w2_sb = proj.tile([128, KT, D], F16)
                b1_sb = proj.tile([128, KT], F32)
                b2_sb = proj.tile([128, KT], F32)
                e_sb = proj.tile([128, KT, NB], F16)
                hsq = proj.tile([128, KT, NB], F32R)

                nc.sync.dma_start(zt_sb[:], zt1)
                nc.sync.dma_start(w1_sb[:], w1t)
                nc.sync.dma_start(w2_sb[:], w2t)
                nc.sync.dma_start(b1_sb[:], b1)
                nc.sync.dma_start(b2_sb[:], b2)

                for v in range(2):
                    if v == 1:
                        nc.sync.dma_start(zt_sb[:], zt2)
                    # ---- layer 1 + ELU ----
                    for pt in range(KT):
                        for ch in range(NB // NCHUNK):
                            ps = ppsum.tile([128, NCHUNK], F32, tag="ps_proj")
                            for kt in range(KT):
                                nc.tensor.matmul(
                                    ps[:],
                                    lhsT=w1_sb[:, kt, pt * 128:(pt + 1) * 128],
                                    rhs=zt_sb[:, kt,
                                              ch * NCHUNK:(ch + 1) * NCHUNK],
                                    start=(kt == 0), stop=(kt == KT - 1))
                            # elu(y) = max(y,0) + min(exp(y),1) - 1,  y = ps + b1
                            texp = ptmp.tile([128, NCHUNK], F16, tag="texp")
                            nc.scalar.activation(texp[:], ps[:], AF.Exp,
                                                 bias=b1_sb[:, pt:pt + 1],
                                                 scale=1.0)
                            tclip = ptmp.tile([128, NCHUNK], F16, tag="tclip")
                            nc.vector.tensor_scalar(tclip[:], texp[:], 1.0, -1.0,
                                                    ALU.min, ALU.add)
                            tmax = ptmp.tile([128, NCHUNK], F16, tag="tmax")
                            nc.scalar.activation(tmax[:], ps[:], AF.Relu,
                                                 bias=b1_sb[:, pt:pt + 1],
                                                 scale=1.0)
                            nc.vector.tensor_tensor(
                                e_sb[:, pt, ch * NCHUNK:(ch + 1) * NCHUNK],
                                tmax[:], tclip[:], ALU.add)
                    # ---- layer 2 (+ b2 on DVE), squares on DVE ----
                    for jt in range(KT):
                        for ch in range(NB // NCHUNK):
                            ps = ppsum.tile([128, NCHUNK], F32, tag="ps_proj")
                            for kt in range(KT):
                                nc.tensor.matmul(
                                    ps[:],
                                    lhsT=w2_sb[:, kt, jt * 128:(jt + 1) * 128],
                                    rhs=e_sb[:, kt, ch * NCHUNK:(ch + 1) * NCHUNK],
                                    start=(kt == 0), stop=(kt == KT - 1))
                            sl = (slice(None), jt,
                                  slice(ch * NCHUNK, (ch + 1) * NCHUNK))
                            nc.vector.tensor_scalar(h_sb[v][sl], ps[:],
                                                    b2_sb[:, jt:jt + 1], None,
                                                    ALU.add)
                            nc.scalar.activation(hsq[sl], h_sb[v][sl], AF.Square)
                    # ---- per-node 1/norm: rn = exp(-0.5*ln(ss)).  Batch the
                    # Ln's then the Exp's so the ACT table switches only twice
                    # per view instead of per chunk.
                    tlns = []
                    for ch in range(NB // NCHUNK):
                        csl = slice(ch * NCHUNK, (ch + 1) * NCHUNK)
                        pss = spsum.tile([1, NCHUNK], F32, tag="ps_small")
                        for jt in range(KT):
                            nc.tensor.matmul(
                                pss[:],
                                lhsT=ones_col[:],
                                rhs=hsq[:, jt, csl],
                                start=(jt == 0), stop=(jt == KT - 1))
                        tln = ptmp.tile([1, NCHUNK], F32, tag="tln")
                        nc.scalar.activation(tln[:], pss[:], AF.Ln)
                        tlns.append(tln)
                    for ch in range(NB // NCHUNK):
                        csl = slice(ch * NCHUNK, (ch + 1) * NCHUNK)
                        nc.scalar.activation(rn_vec[v][:, csl], tlns[ch][:],
                                             AF.Exp, scale=-0.5)
                    for ch in range(NB // NCHUNK):
                        csl = slice(ch * NCHUNK, (ch + 1) * NCHUNK)
                        # broadcast rn across partitions (K=1 ones-matmul)
                        pbc = spsum.tile([128, NCHUNK], F32, tag="ps_bc")
                        nc.tensor.matmul(
                            pbc[:], lhsT=ones_row[:],
                            rhs=rn_vec[v][:, csl],
                            start=True, stop=True)
                        for jt in range(KT):
                            nc.vector.tensor_tensor(
                                n8[v][:, jt, csl], h_sb[v][:, jt, csl],
                                pbc[:], ALU.mult)

                    # ship to DRAM + AllGather (fp8; overlaps with the other
                    # view / the sims).  Per-kt DMAs: the first slice ships
                    # while later kt's quant is still running.
                    for kt in range(KT):
                        nc.sync.dma_start(
                            cc_in[v][kt * 128:(kt + 1) * 128, :],
                            n8[v][:, kt, :])
                    if sim_mode:
                        nc.sync.dma_start(cc_out[v][0:D, :], cc_in[v][:])
                    else:
                        nc.gpsimd.collective_compute(
                            "AllGather", ALU.bypass, replica_groups=rg,
                            ins=[cc_in[v].opt()], outs=[cc_out[v].opt()])

                # ---- pos diagonal: s12_ii = rn1_i*rn2_i*sum_f h1[f,i]h2[f,i]
                hh = hsq  # reuse
                for jt in range(KT):
                    nc.vector.tensor_tensor(hh[:, jt, :], h_sb[0][:, jt, :],
                                            h_sb[1][:, jt, :], ALU.mult)
                for ch in range(NB // NCHUNK):
                    csl = slice(ch * NCHUNK, (ch + 1) * NCHUNK)
                    psp = spsum.tile([1, NCHUNK], F32, tag="ps_small")
                    for jt in range(KT):
                        nc.tensor.matmul(psp[:],
                                         lhsT=ones_col[:],
                                         rhs=hh[:, jt, csl],
                                         start=(jt == 0), stop=(jt == KT - 1))
                    nc.vector.tensor_tensor(pos_part[:, csl], psp[:],
                                            rn_vec[0][:, csl], ALU.mult)
                    nc.vector.tensor_tensor(pos_part[:, csl], pos_part[:, csl],
                                            rn_vec[1][:, csl], ALU.mult)
                nc.vector.tensor_reduce(pos_sum[:], pos_part[:],
                                        mybir.AxisListType.X, ALU.add)

            # ---------------- load gathered embeddings ----------------
            for v in range(2):
                for r in range(N_CORES):
                    nc.sync.dma_start(
                        g_sb[v][:, :, r * NB:(r + 1) * NB],
                        cc_out[v][r * D:(r + 1) * D, :]
                        .rearrange("(ko p) m -> p ko m", p=128))

            # ---------------- sims: S11 then S12 ----------------
            # fp8 DoubleRow, K=256 per step.  [128, 2048] psum groups,
            # exp(2s) + row sums on ACT; S12's exp tiles also accumulate
            # into `acc` (DVE) for the S21 row sums (colsums of exp(S12)).
            def sim_pass(x, vl, vr, pool, scr, do_acc):
                for mt in range(MT):
                    for g in range(NG):
                        pss = pool.tile([128, GW], F32, tag="ps_sim")
                        for kt2 in range(KT // 2):
                            for ch in range(GW // NCHUNK):
                                c0 = g * GW + ch * NCHUNK
                                nc.tensor.matmul(
                                    pss[:, ch * NCHUNK:(ch + 1) * NCHUNK],
                                    lhsT=n8[vl][:, 2 * kt2:2 * kt2 + 2,
                                                mt * 128:(mt + 1) * 128],
                                    rhs=g_sb[vr][:, 2 * kt2:2 * kt2 + 2,
                                                 c0:c0 + NCHUNK],
                                    start=(kt2 == 0), stop=(kt2 == KT // 2 - 1),
                                    perf_mode=DR)
                        es = scr.tile([128, GW], F16, tag="es")
                        nc.scalar.activation(
                            es[:], pss[:], AF.Exp, scale=TAU_INV,
                            accum_out=parts[x][:, mt, g:g + 1])
                        if do_acc:
                            asl = acc[:, g * GW:(g + 1) * GW]
                            if mt == 0:
                                nc.vector.tensor_copy(asl, es[:])
                            else:
                                nc.vector.tensor_tensor(asl, asl, es[:],
                                                        ALU.add)

            with tc.tile_pool(name="sim_psum", bufs=2, space="PSUM") as sp, \
                 tc.tile_pool(name="scr", bufs=4) as scr, \
                 tc.tile_pool(name="cs_sbp", bufs=4) as cs_sbp:
                sim_pass(0, 0, 0, sp, scr, False)   # S11
                sim_pass(1, 0, 1, sp, scr, True)    # S12 (+ colsum acc)

                # ---- S22, with the S21 colsum reduction interleaved.
                # After row-tiles 2..5 of S22, borrow one sim-psum ring slot
                # for 4 ones-matmul partition reductions of `acc`; DVE copies
                # them out and small DMAs stream them to cs_in.  The
                # ReduceScatter then overlaps the tail of S22.
                for mt in range(MT):
                    for g in range(NG):
                        pss = sp.tile([128, GW], F32, tag="ps_sim")
                        for kt2 in range(KT // 2):
                            for ch in range(GW // NCHUNK):
                                c0 = g * GW + ch * NCHUNK
                                nc.tensor.matmul(
                                    pss[:, ch * NCHUNK:(ch + 1) * NCHUNK],
                                    lhsT=n8[1][:, 2 * kt2:2 * kt2 + 2,
                                               mt * 128:(mt + 1) * 128],
                                    rhs=g_sb[1][:, 2 * kt2:2 * kt2 + 2,
                                                c0:c0 + NCHUNK],
                                    start=(kt2 == 0), stop=(kt2 == KT // 2 - 1),
                                    perf_mode=DR)
                        es = scr.tile([128, GW], F16, tag="es")
                        nc.scalar.activation(
                            es[:], pss[:], AF.Exp, scale=TAU_INV,
                            accum_out=parts[2][:, mt, g:g + 1])
                    if mt <= 3:
                        rnd = mt
                        pcol = sp.tile([128, GW], F32, tag="ps_sim")
                        for i in range(4):
                            c = rnd * 4 + i
                            nc.tensor.matmul(
                                pcol[0:1, i * NCHUNK:(i + 1) * NCHUNK],
                                lhsT=ones_16[:],
                                rhs=acc[:, c * NCHUNK:(c + 1) * NCHUNK],
                                start=True, stop=True)
                        for i in range(4):
                            c = rnd * 4 + i
                            cst = cs_sbp.tile([1, NCHUNK], F32, tag="cs")
                            nc.vector.tensor_copy(
                                cst[:], pcol[0:1, i * NCHUNK:(i + 1) * NCHUNK])
                            nc.sync.dma_start(
                                cs_in[c * NCHUNK:(c + 1) * NCHUNK], cst[:])
                    if mt == 3:
                        if sim_mode:
                            nc.sync.dma_start(cs_out[:], cs_in[0:NB])
                        else:
                            nc.gpsimd.collective_compute(
                                "ReduceScatter", ALU.add, replica_groups=rg,
                                ins=[cs_in.opt()], outs=[cs_out.opt()])
                        nc.sync.dma_start(
                            rs21[:],
                            cs_out.rearrange("(mt p) -> p mt", p=128))

            # ---------------- assemble the loss ----------------
            with tc.tile_pool(name="fin", bufs=1) as fsb, \
                 tc.tile_pool(name="fin_psum", bufs=1, space="PSUM") as fp:
                for x in range(3):
                    nc.vector.tensor_reduce(rs[x][:], parts[x][:],
                                            mybir.AxisListType.X, ALU.add)
                d1 = fsb.tile([128, MT], F32)
                d2 = fsb.tile([128, MT], F32)
                nc.vector.tensor_tensor(d1[:], rs[0][:], rs[1][:], ALU.add)
                nc.vector.tensor_scalar_add(d1[:], d1[:], -E2)
                nc.vector.tensor_tensor(d2[:], rs[2][:], rs21[:], ALU.add)
                nc.vector.tensor_scalar_add(d2[:], d2[:], -E2)
                nc.scalar.activation(d1[:], d1[:], AF.Ln)
                nc.scalar.activation(d2[:], d2[:], AF.Ln)
                lsum = fsb.tile([128, MT], F32)
                nc.vector.tensor_tensor(lsum[:], d1[:], d2[:], ALU.add)
                lrow = fsb.tile([128, 1], F32)
                nc.vector.tensor_reduce(lrow[:], lsum[:],
                                        mybir.AxisListType.X, ALU.add)
                pfin = fp.tile([1, 1], F32)
                nc.tensor.matmul(pfin[:], lhsT=ones_cs[:], rhs=lrow[:],
                                 start=True, stop=True)
                fin = fsb.tile([1, 1], F32)
                nc.vector.tensor_scalar_mul(fin[:], pfin[:], 0.5)
                p2 = fsb.tile([1, 1], F32)
                nc.vector.tensor_scalar_mul(p2[:], pos_sum[:], 2.0)
                nc.vector.tensor_tensor(fin[:], fin[:], p2[:], ALU.subtract)
                nc.sync.dma_start(out, fin[:])

    nc.compile()
    return nc


def _prep_inputs(z1, z2, fc1_w, fc1_b, fc2_w, fc2_b):
    """Host-side shard + layout prep. Returns in_maps for the 8 cores."""
    w1t = np.ascontiguousarray(fc1_w.T).reshape(KT, 128, D).transpose(1, 0, 2)
    w1t = np.ascontiguousarray(w1t, dtype=np.float16)
    w2t = np.ascontiguousarray(fc2_w.T).reshape(KT, 128, D).transpose(1, 0, 2)
    w2t = np.ascontiguousarray(w2t, dtype=np.float16)
    b1 = np.ascontiguousarray(fc1_b.reshape(KT, 128).T, dtype=np.float32)
    b2 = np.ascontiguousarray(fc2_b.reshape(KT, 128).T, dtype=np.float32)

    in_maps = []
    for c in range(N_CORES):
        blk1 = z1[c * NB:(c + 1) * NB].T            # [512, 1024]
        blk2 = z2[c * NB:(c + 1) * NB].T
        zt1 = np.ascontiguousarray(
            blk1.reshape(KT, 128, NB).transpose(1, 0, 2), dtype=np.float16)
        zt2 = np.ascontiguousarray(
            blk2.reshape(KT, 128, NB).transpose(1, 0, 2), dtype=np.float16)
        in_maps.append({"zt1": zt1, "zt2": zt2, "w1t": w1t, "w2t": w2t,
                        "b1": b1, "b2": b2})
    return in_maps


def kernel(z1, z2, fc1_w, fc1_b, fc2_w, fc2_b):
    global LAST_EXEC_NS
    z1 = np.asarray(z1, dtype=np.float32)
    z2 = np.asarray(z2, dtype=np.float32)
    fc1_w = np.asarray(fc1_w, dtype=np.float32)
    fc1_b = np.asarray(fc1_b, dtype=np.float32)
    fc2_w = np.asarray(fc2_w, dtype=np.float32)
    fc2_b = np.asarray(fc2_b, dtype=np.float32)

    if "nc" not in _CACHE:
        _CACHE["nc"] = _build_program()
    nc = _CACHE["nc"]

    in_maps = _prep_inputs(z1, z2, fc1_w, fc1_b, fc2_w, fc2_b)
    res = run_bass_kernel_spmd(nc, in_maps, core_ids=list(range(N_CORES)),
                               trace=TRACE)
    LAST_EXEC_NS = res.exec_time_ns
    total = math.fsum(float(r["out"][0, 0]) for r in res.results)
    return np.float32(total / N)
